# revision 10
# baseline (speedup 1.0000x reference)
"""Distributed Trainium2 Bass kernel for nn_Attention_13125420057022.

Multi-head attention (B=2, S=2048, H=768, 12 heads, head_dim=64) with
interleaved RoPE, run SPMD on 8 NeuronCores.

Sharding: core c handles batch b=c//4 and query rows [512*(c%4), 512*(c%4+1)).
Each core computes QKV for its own 512 sequence rows, applies RoPE, then
AllGathers (replica groups [[0..3],[4..7]]) K^T and V for the full sequence of
its batch. Attention and the output projection then produce a disjoint slice
of the output rows, so no further collective is needed.

The gathers (4-rank ring bandwidth, ~45us each for 3MB) are hidden behind
compute: K^T is gathered first (issued as soon as the K blocks finish), then
V; while they are in flight each core runs Q-block QKV+RoPE and the attention
blocks against its OWN K/V quarter (still in SBUF), saving per-head partial
context+sums. Foreign scores can start as soon as the K gather lands; the A.V
stage waits only for the V gather. The gathered read-back offsets depend on
the core's position within its replica group, so those DMAs sit in 4-way If
trees on a per-core `jid` input.

Compute is bf16 with f32 PSUM accumulation. Scores are computed transposed
(S^T[k,q] = sum_d K^T[d,k] Q^T[d,q]) so the exp() output feeds the A.V matmul
directly with no on-chip transposes. Softmax row-sums come from a ones column
appended to V; no max-subtraction is needed because the logits are bounded.
The per-query 1/sum broadcast runs on GpSimd (partition_broadcast). RoPE uses
a host-side de-interleave permutation of the Q/K weight rows to turn
interleaved rotation into rotate-half form (contiguous 32-row block swaps).
"""

import math
import sys
from contextlib import ExitStack

import numpy as np
import ml_dtypes

sys.path.insert(0, "/opt/trn_rl_repo")

import concourse.bass as bass  # noqa: E402
import concourse.mybir as mybir  # noqa: E402
import concourse.tile as tile  # noqa: E402
from concourse import bacc  # noqa: E402
from concourse.bass_utils import run_bass_kernel_spmd  # noqa: E402

BF16 = ml_dtypes.bfloat16
F32 = mybir.dt.float32
BF = mybir.dt.bfloat16

B, S, H = 2, 2048, 768
NH, HD = 12, 64
THETA = 10000.0
NCORES = 8
GROUP = 4  # cores per batch
SLOC = S // GROUP  # 512 query rows per core
NKB_OWN = SLOC // 128  # 4 own key blocks
NKB_FOR = 3 * NKB_OWN  # 12 foreign key blocks
REPLICA_GROUPS = [[0, 1, 2, 3], [4, 5, 6, 7]]

K_ELEMS = H * SLOC  # K^T local: [768, 512]
V_ELEMS = SLOC * H  # V local: [512, 768]

EXP = mybir.ActivationFunctionType.Exp
COPY = mybir.ActivationFunctionType.Copy
IDENT = mybir.ActivationFunctionType.Identity


def build_graph():
    nc = bacc.Bacc(
        "TRN2",
        target_bir_lowering=False,
        debug=False,
        num_devices=NCORES,
    )

    # External inputs (per-core shards, host-prepped)
    xT = nc.dram_tensor("xT", [H, SLOC], BF, kind="ExternalInput")
    wt = nc.dram_tensor("wt", [H, 3 * H], BF, kind="ExternalInput")
    qkvb_qk = nc.dram_tensor("qkvb_qk", [128, 12], F32, kind="ExternalInput")
    qkvb_v = nc.dram_tensor("qkvb_v", [1, H], BF, kind="ExternalInput")
    projt = nc.dram_tensor("projt", [H, H], BF, kind="ExternalInput")
    projb = nc.dram_tensor("projb", [1, H], BF, kind="ExternalInput")
    cq = nc.dram_tensor("cq", [128, SLOC], BF, kind="ExternalInput")
    sq = nc.dram_tensor("sq", [128, SLOC], BF, kind="ExternalInput")
    ck = nc.dram_tensor("ck", [128, SLOC], BF, kind="ExternalInput")
    sk = nc.dram_tensor("sk", [128, SLOC], BF, kind="ExternalInput")
    jid = nc.dram_tensor("jid", [1, 1], mybir.dt.uint32, kind="ExternalInput")
    out_ext = nc.dram_tensor("out", [SLOC, H], F32, kind="ExternalOutput")

    # Collective bounce buffers (internal DRAM). Shared addr_space is
    # rejected for 4-core replica groups; Local works.
    k_local = nc.dram_tensor("k_local", [K_ELEMS], BF)
    k_g = nc.dram_tensor("k_g", [GROUP, K_ELEMS], BF)
    v_local = nc.dram_tensor("v_local", [V_ELEMS], BF)
    v_g = nc.dram_tensor("v_g", [GROUP, V_ELEMS], BF)

    kl_r = k_local.ap().rearrange("(t p s) -> t p s", t=6, p=128, s=SLOC)
    vl_r = v_local.ap().rearrange("(i p n) -> i p n", i=4, p=128, n=H)

    def k_g_r(r, t):
        return k_g.ap()[r].rearrange("(t p s) -> t p s", t=6, p=128, s=SLOC)[t]

    def v_g_r(r, i):
        return v_g.ap()[r].rearrange("(i p h d) -> i p h d", i=4, p=128, h=NH, d=HD)[i]

    with tile.TileContext(nc) as tc, ExitStack() as ctx:
        singles = ctx.enter_context(tc.tile_pool(name="singles", bufs=1))
        qk_raw_p = ctx.enter_context(tc.tile_pool(name="qk_raw", bufs=3))
        qk_swp_p = ctx.enter_context(tc.tile_pool(name="qk_swp", bufs=3))
        rope_tmp_p = ctx.enter_context(tc.tile_pool(name="rope_tmp", bufs=3))
        k_own_p = ctx.enter_context(tc.tile_pool(name="k_own", bufs=6))
        v_cc_p = ctx.enter_context(tc.tile_pool(name="v_cc", bufs=3))
        v_pool = ctx.enter_context(tc.tile_pool(name="v_pool", bufs=1))
        own_ctx_p = ctx.enter_context(tc.tile_pool(name="own_ctx", bufs=1))
        at_pool = ctx.enter_context(tc.tile_pool(name="at", bufs=8))
        small_p = ctx.enter_context(tc.tile_pool(name="small", bufs=2))
        ctxn_p = ctx.enter_context(tc.tile_pool(name="ctxn", bufs=2))
        out_p = ctx.enter_context(tc.tile_pool(name="outp", bufs=2))

        # ---- SBUF tiles ----
        wt_sb = singles.tile([128, 6, 3 * H], BF)
        xT_sb = singles.tile([128, 6, SLOC], BF)
        projt_sb = singles.tile([128, 6, H], BF)
        projb_sb = singles.tile([1, H], BF)
        qkvb_sb = singles.tile([128, 12], F32)
        qkvbv_sb = singles.tile([1, H], BF)
        cq_sb = singles.tile([128, SLOC], BF)
        sq_sb = singles.tile([128, SLOC], BF)
        ck_sb = singles.tile([128, SLOC], BF)
        sk_sb = singles.tile([128, SLOC], BF)
        ones_bf = singles.tile([1, 128], BF)
        qT_sb = singles.tile([128, 6, SLOC], BF)
        kTf_sb = singles.tile([128, 6, 3 * SLOC], BF)  # foreign K^T sections
        ctxT_sb = singles.tile([128, 6, SLOC], BF)

        wt_r = wt.ap().rearrange("(c p) n -> c p n", p=128)
        xT_r = xT.ap().rearrange("(c p) s -> c p s", p=128)
        projt_r = projt.ap().rearrange("(c p) n -> c p n", p=128)
        # load order: x first, then K columns of W, then V, then Q — so the
        # K blocks (which feed the first collective) can start ASAP.
        for c in range(6):
            nc.sync.dma_start(out=xT_sb[:, c, :], in_=xT_r[c])
        nc.sync.dma_start(out=qkvb_sb[:], in_=qkvb_qk.ap())
        for c in range(6):
            nc.sync.dma_start(out=wt_sb[:, c, 768:1536], in_=wt_r[c][:, 768:1536])
        for c in range(6):
            nc.sync.dma_start(out=wt_sb[:, c, 1536:2304], in_=wt_r[c][:, 1536:2304])
        nc.sync.dma_start(out=qkvbv_sb[:], in_=qkvb_v.ap())
        for c in range(6):
            nc.sync.dma_start(out=wt_sb[:, c, 0:768], in_=wt_r[c][:, 0:768])
        nc.scalar.dma_start(out=ck_sb[:], in_=ck.ap())
        nc.scalar.dma_start(out=sk_sb[:], in_=sk.ap())
        nc.scalar.dma_start(out=cq_sb[:], in_=cq.ap())
        nc.scalar.dma_start(out=sq_sb[:], in_=sq.ap())
        for c in range(6):
            nc.scalar.dma_start(out=projt_sb[:, c, :], in_=projt_r[c])
        nc.scalar.dma_start(out=projb_sb[:], in_=projb.ap())
        nc.vector.memset(ones_bf[:], 1.0)

        with tc.tile_pool(name="st_ps", bufs=5, space="PSUM") as st_psum:

            def qk_block(t, is_q):
                """One 128-row block of Q^T/K^T (heads 2t,2t+1): bias + rope."""
                nb = t + (0 if is_q else 6)  # wt column block (q first, then k)
                ps = st_psum.tile([128, SLOC], F32, tag="st")
                for c in range(6):
                    nc.tensor.matmul(
                        ps[:],
                        lhsT=wt_sb[:, c, nb * 128 : (nb + 1) * 128],
                        rhs=xT_sb[:, c, :],
                        start=(c == 0),
                        stop=(c == 5),
                    )
                raw = qk_raw_p.tile([128, SLOC], BF)
                nc.scalar.activation(
                    out=raw[:], in_=ps[:], func=IDENT, bias=qkvb_sb[:, nb : nb + 1]
                )
                swp = qk_swp_p.tile([128, SLOC], BF)
                nc.scalar.dma_start(out=swp[0:32, :], in_=raw[32:64, :])
                nc.scalar.dma_start(out=swp[32:64, :], in_=raw[0:32, :])
                nc.scalar.dma_start(out=swp[64:96, :], in_=raw[96:128, :])
                nc.scalar.dma_start(out=swp[96:128, :], in_=raw[64:96, :])
                cos_sb, sin_sb = (cq_sb, sq_sb) if is_q else (ck_sb, sk_sb)
                t1 = rope_tmp_p.tile([128, SLOC], BF, tag="t1")
                t2 = rope_tmp_p.tile([128, SLOC], BF, tag="t2")
                nc.vector.tensor_mul(t1[:], raw[:], cos_sb[:])
                nc.vector.tensor_mul(t2[:], swp[:], sin_sb[:])
                if is_q:
                    nc.vector.tensor_add(qT_sb[:, t, :], t1[:], t2[:])
                    return None
                kt = k_own_p.tile([128, SLOC], BF, tag="kown")
                nc.vector.tensor_add(kt[:], t1[:], t2[:])
                return kt

            # ---- K blocks -> bounce, then gather K immediately ----
            kts = []
            for t in range(6):
                kt = qk_block(t, is_q=False)
                nc.sync.dma_start(out=kl_r[t], in_=kt[:])
                kts.append(kt)
            nc.gpsimd.collective_compute(
                "AllGather",
                mybir.AluOpType.bypass,
                replica_groups=REPLICA_GROUPS,
                ins=[k_local.ap().opt()],
                outs=[k_g.ap().opt()],
            )

            # ---- V blocks (natural layout) -> bounce + own V' tiles ----
            vown_tiles = []
            with tc.tile_pool(name="v_ps", bufs=1, space="PSUM") as v_psum:
                for i in range(4):
                    vps = v_psum.tile([128, H], F32)
                    for c in range(6):
                        lhsT = xT_sb[:, c, i * 128 : (i + 1) * 128]
                        nc.tensor.matmul(
                            vps[:, 0:512],
                            lhsT=lhsT,
                            rhs=wt_sb[:, c, 1536:2048],
                            start=(c == 0),
                            stop=False,
                        )
                        nc.tensor.matmul(
                            vps[:, 512:768],
                            lhsT=lhsT,
                            rhs=wt_sb[:, c, 2048:2304],
                            start=(c == 0),
                            stop=False,
                        )
                    # bias via ones-row (rank-1 update), also closes the groups
                    nc.tensor.matmul(
                        vps[:, 0:512],
                        lhsT=ones_bf[:, 0:128],
                        rhs=qkvbv_sb[:, 0:512],
                        start=False,
                        stop=True,
                    )
                    nc.tensor.matmul(
                        vps[:, 512:768],
                        lhsT=ones_bf[:, 0:128],
                        rhs=qkvbv_sb[:, 512:768],
                        start=False,
                        stop=True,
                    )
                    vsb = v_cc_p.tile([128, H], BF)
                    nc.scalar.activation(out=vsb[:], in_=vps[:], func=COPY)
                    nc.sync.dma_start(out=vl_r[i], in_=vsb[:])
                    # own V' tile (with ones column) from the SBUF copy
                    vt = v_pool.tile([128, NH * 65], BF, tag=f"vo{i}")
                    vt3 = vt.rearrange("p (h c) -> p h c", h=NH)
                    nc.scalar.dma_start(
                        out=vt3[:, :, 0:64],
                        in_=vsb.rearrange("p (h d) -> p h d", h=NH),
                    )
                    nc.vector.memset(vt3[:, :, 64:65], 1.0)
                    vown_tiles.append(vt)
            nc.gpsimd.collective_compute(
                "AllGather",
                mybir.AluOpType.bypass,
                replica_groups=REPLICA_GROUPS,
                ins=[v_local.ap().opt()],
                outs=[v_g.ap().opt()],
            )

            # ---- Q blocks (overlap with the collectives) ----
            for t in range(6):
                qk_block(t, is_q=True)

            # ---- foreign K^T read-back (position-dependent, on sync) ----
            jreg_s = nc.sync.alloc_register("jid_sync")
            nc.sync.reg_load(jreg_s, jid.ap()[0:1, 0:1])
            jv_s = nc.sync.snap(jreg_s, donate=True, min_val=0, max_val=3)

            def k_leaf(j):
                fsecs = [r for r in range(GROUP) if r != j]
                for t in range(6):
                    for s, r in enumerate(fsecs):
                        nc.sync.dma_start(
                            out=kTf_sb[:, t, s * SLOC : (s + 1) * SLOC],
                            in_=k_g_r(r, t),
                        )

            with tc.If(jv_s < 2) as c1:
                with tc.If(jv_s < 1) as c2:
                    k_leaf(0)
                with c2.Else():
                    k_leaf(1)
            with c1.Else():
                with tc.If(jv_s < 3) as c3:
                    k_leaf(2)
                with c3.Else():
                    k_leaf(3)

            with (
                tc.tile_pool(name="ctx_ps", bufs=2, space="PSUM") as ctx_psum,
            ):
                # ---- own-block attention pass (overlaps the collectives) ----
                own_sbs = []
                for h in range(NH):
                    t, r0 = h // 2, (h % 2) * 64
                    cop = ctx_psum.tile([65, SLOC], F32, tag="ctx")
                    for kb in range(NKB_OWN):
                        st = st_psum.tile([128, SLOC], F32, tag="st")
                        nc.tensor.matmul(
                            st[:],
                            lhsT=kts[t][r0 : r0 + 64, kb * 128 : (kb + 1) * 128],
                            rhs=qT_sb[r0 : r0 + 64, t, :],
                            start=True,
                            stop=True,
                        )
                        at = at_pool.tile([128, SLOC], BF)
                        nc.scalar.activation(out=at[:], in_=st[:], func=EXP)
                        nc.tensor.matmul(
                            cop[:],
                            lhsT=vown_tiles[kb][:, h * 65 : (h + 1) * 65],
                            rhs=at[:],
                            start=(kb == 0),
                            stop=(kb == NKB_OWN - 1),
                        )
                    osb = own_ctx_p.tile([65, SLOC], BF, tag=f"own{h}")
                    nc.vector.tensor_copy(osb[:], cop[:])
                    own_sbs.append(osb)

                # ---- foreign V read-back (on scalar queue) ----
                vfor_tiles = []
                for kb in range(NKB_FOR):
                    vt = v_pool.tile([128, NH * 65], BF, tag=f"vf{kb}")
                    nc.vector.memset(
                        vt.rearrange("p (h c) -> p h c", h=NH)[:, :, 64:65], 1.0
                    )
                    vfor_tiles.append(vt)

                jreg_a = nc.scalar.alloc_register("jid_scalar")
                nc.scalar.reg_load(jreg_a, jid.ap()[0:1, 0:1])
                jv_a = nc.scalar.snap(jreg_a, donate=True, min_val=0, max_val=3)

                def v_leaf(j):
                    fsecs = [r for r in range(GROUP) if r != j]
                    for s, r in enumerate(fsecs):
                        for i in range(4):
                            vt3 = vfor_tiles[s * 4 + i].rearrange(
                                "p (h c) -> p h c", h=NH
                            )
                            nc.scalar.dma_start(
                                out=vt3[:, :, 0:64], in_=v_g_r(r, i)
                            )

                with tc.If(jv_a < 2) as d1:
                    with tc.If(jv_a < 1) as d2:
                        v_leaf(0)
                    with d2.Else():
                        v_leaf(1)
                with d1.Else():
                    with tc.If(jv_a < 3) as d3:
                        v_leaf(2)
                    with d3.Else():
                        v_leaf(3)

                # ---- foreign attention pass + pipelined epilogue ----
                def head_mm(h):
                    t, r0 = h // 2, (h % 2) * 64
                    ctxp = ctx_psum.tile([65, SLOC], F32, tag="ctx")
                    for kb in range(NKB_FOR):
                        st = st_psum.tile([128, SLOC], F32, tag="st")
                        nc.tensor.matmul(
                            st[:],
                            lhsT=kTf_sb[r0 : r0 + 64, t, kb * 128 : (kb + 1) * 128],
                            rhs=qT_sb[r0 : r0 + 64, t, :],
                            start=True,
                            stop=True,
                        )
                        at = at_pool.tile([128, SLOC], BF)
                        nc.scalar.activation(out=at[:], in_=st[:], func=EXP)
                        nc.tensor.matmul(
                            ctxp[:],
                            lhsT=vfor_tiles[kb][:, h * 65 : (h + 1) * 65],
                            rhs=at[:],
                            start=(kb == 0),
                            stop=(kb == NKB_FOR - 1),
                        )
                    return ctxp

                def head_epilogue(h, ctxp):
                    t, r0 = h // 2, (h % 2) * 64
                    osb = own_sbs[h]
                    # sums row (partition 64): foreign + own, lane-aligned
                    sums64 = small_p.tile([128, SLOC], F32, tag="sums64")
                    nc.vector.tensor_add(
                        sums64[64:65, :], ctxp[64:65, :], osb[64:65, :]
                    )
                    sums = small_p.tile([1, SLOC], F32, tag="sums")
                    nc.scalar.dma_start(out=sums[:], in_=sums64[64:65, :])
                    rec = small_p.tile([1, SLOC], F32, tag="rec")
                    nc.vector.reciprocal_approx_fast(out=rec[:], in_=sums[:])
                    bc_sb = small_p.tile([64, SLOC], F32, tag="bc_sb")
                    nc.gpsimd.partition_broadcast(bc_sb[:], rec[:], channels=64)
                    ctmp = small_p.tile([64, SLOC], F32, tag="ctmp")
                    nc.vector.tensor_add(ctmp[:], ctxp[0:64, :], osb[0:64, :])
                    if r0 == 0:
                        nc.vector.tensor_mul(ctxT_sb[0:64, t, :], ctmp[:], bc_sb[:])
                    else:
                        cn = ctxn_p.tile([64, SLOC], BF)
                        nc.vector.tensor_mul(cn[:], ctmp[:], bc_sb[:])
                        nc.scalar.dma_start(out=ctxT_sb[64:128, t, :], in_=cn[:])

                prev = None
                for h in range(NH):
                    ctxp = head_mm(h)
                    if prev is not None:
                        head_epilogue(h - 1, prev)
                    prev = ctxp
                head_epilogue(NH - 1, prev)

        # ---- output projection: out[s,:] = ctx^T.T @ projt + projb ----
        with tc.tile_pool(name="o_ps", bufs=2, space="PSUM") as o_psum:
            for i in range(4):
                ops = o_psum.tile([128, H], F32)
                for c in range(6):
                    lhsT = ctxT_sb[:, c, i * 128 : (i + 1) * 128]
                    nc.tensor.matmul(
                        ops[:, 0:512],
                        lhsT=lhsT,
                        rhs=projt_sb[:, c, 0:512],
                        start=(c == 0),
                        stop=False,
                    )
                    nc.tensor.matmul(
                        ops[:, 512:768],
                        lhsT=lhsT,
                        rhs=projt_sb[:, c, 512:768],
                        start=(c == 0),
                        stop=False,
                    )
                nc.tensor.matmul(
                    ops[:, 0:512],
                    lhsT=ones_bf[:, 0:128],
                    rhs=projb_sb[:, 0:512],
                    start=False,
                    stop=True,
                )
                nc.tensor.matmul(
                    ops[:, 512:768],
                    lhsT=ones_bf[:, 0:128],
                    rhs=projb_sb[:, 512:768],
                    start=False,
                    stop=True,
                )
                osb = out_p.tile([128, H], F32)
                nc.scalar.activation(out=osb[:], in_=ops[:], func=COPY)
                nc.sync.dma_start(
                    out=out_ext.ap()[i * 128 : (i + 1) * 128, :], in_=osb[:]
                )

    nc.compile()
    return nc


_PERM = np.concatenate([np.arange(0, HD, 2), np.arange(1, HD, 2)])


def prep_inputs(x, qkv_w, qkv_b, proj_w, proj_b):
    """Shard + lay out the full inputs into per-core input maps."""
    x = np.asarray(x, np.float32)
    qkv_w = np.asarray(qkv_w, np.float32)
    qkv_b = np.asarray(qkv_b, np.float32)
    proj_w = np.asarray(proj_w, np.float32)
    proj_b = np.asarray(proj_b, np.float32)

    # de-interleave permutation of q/k head dims (rows of qkv_w)
    Wp = qkv_w.copy()
    bp = qkv_b.copy()
    for sec in range(2):
        for h in range(NH):
            base = sec * H + h * HD
            Wp[base : base + HD] = qkv_w[base + _PERM]
            bp[base : base + HD] = qkv_b[base + _PERM]
    wt = np.ascontiguousarray(Wp.T).astype(BF16)  # [768, 2304]
    qkvb_qk = np.ascontiguousarray(bp[: 2 * H].reshape(12, 128).T).astype(np.float32)
    qkvb_v = qkv_b[2 * H :].reshape(1, H).astype(BF16)
    projt = np.ascontiguousarray(proj_w.T).astype(BF16)
    projb = proj_b.reshape(1, H).astype(BF16)

    inv_freq = 1.0 / (THETA ** (np.arange(0, HD, 2, dtype=np.float32) / HD))
    angles = np.arange(S, dtype=np.float32)[None, :] * inv_freq[:, None]  # [32, S]
    cos_t, sin_t = np.cos(angles), np.sin(angles)
    qscale = 1.0 / math.sqrt(HD)

    in_maps = []
    for c in range(NCORES):
        b, j = c // GROUP, c % GROUP
        sl = slice(j * SLOC, (j + 1) * SLOC)
        cos_j, sin_j = cos_t[:, sl], sin_t[:, sl]
        ck_a = np.tile(cos_j, (4, 1)).astype(BF16)
        sk_a = np.concatenate([-sin_j, sin_j, -sin_j, sin_j], axis=0).astype(BF16)
        cq_a = np.tile(cos_j * qscale, (4, 1)).astype(BF16)
        sq_a = np.concatenate(
            [-sin_j * qscale, sin_j * qscale, -sin_j * qscale, sin_j * qscale], axis=0
        ).astype(BF16)
        xT = np.ascontiguousarray(x[b, sl, :].T).astype(BF16)
        in_maps.append(
            {
                "xT": xT,
                "wt": wt,
                "qkvb_qk": qkvb_qk,
                "qkvb_v": qkvb_v,
                "projt": projt,
                "projb": projb,
                "cq": cq_a,
                "sq": sq_a,
                "ck": ck_a,
                "sk": sk_a,
                "jid": np.array([[j]], dtype=np.uint32),
            }
        )
    return in_maps


_NC_CACHE = {}


def get_graph():
    if "nc" not in _NC_CACHE:
        _NC_CACHE["nc"] = build_graph()
    return _NC_CACHE["nc"]


def run(inputs, trace=False, **kw):
    nc = get_graph()
    in_maps = prep_inputs(**inputs)
    res = run_bass_kernel_spmd(nc, in_maps, core_ids=list(range(NCORES)), trace=trace, **kw)
    out = np.empty((B, S, H), np.float32)
    for c in range(NCORES):
        b, j = c // GROUP, c % GROUP
        out[b, j * SLOC : (j + 1) * SLOC, :] = res.results[c]["out"]
    return out, res


def kernel(**inputs):
    out, _ = run(inputs, trace=False)
    return out


if __name__ == "__main__":
    print("building graph...")
    nc = get_graph()
    print("graph built and compiled")
